# revision 33
# baseline (speedup 1.0000x reference)
"""Trainium2 Bass kernel for the cosine-gated LSTM cell (CGLSTMCellv1).

Full inputs in, full outputs out. Internally: data-parallel shard of the
batch dim across 8 NeuronCores, weights replicated, no cross-core comms.

Math per core (rows = local batch slice):
  mapped = x @ Wm + bm
  attn   = sigmoid(cos_sim(mapped, hx));  s = 1 + attn
  gates  = concat(s*x, hx) @ W + b  = s*(x@Wx) + hx@Wh + b  (s folded into xT)
  i,f,g,o = LN-gates -> sigmoid/tanh
  cx_new = f*cx + i*g ; hx_new = o*tanh(cx_new)
  hx_mod = hx_new * (1 + sigmoid((cos_sim(hx_new,cx_new)+1)/2))

Walrus codegen limits sync waits per instruction (Matmult: 1, DMA: 2), so
the kernel is structured to keep semaphore fan-in low: PSUM tiles are
released by a single engine, x/hx are preloaded into dedicated tiles, the
gamma/beta replicas are consumed by gpsimd only, and dummy "absorber"
transposes pre-observe DMA semaphores before real matmuls need them.

All big matmuls run with float32r-bitcast operands: same fp32 bytes, but
the PE streams 1 column/cycle (vs 4 cyc for plain fp32) when N >= 256.
"""

import numpy as np

B_FULL, DIM_I, DIM_H = 8192, 1024, 1024
NCORES = 8
BL = B_FULL // NCORES  # 1024 rows per core
P = 128
H4 = 4 * DIM_H
NKB1 = DIM_I // P           # 8  k-blocks for mm1
NKB2 = (DIM_I + DIM_H) // P  # 16 k-blocks for mm2
CHUNK = 512                 # W column chunk
NCH_G = DIM_H // CHUNK      # 2 chunks per gate
ZW = (NCH_G - 1) * CHUNK    # staging width for all but the last chunk
LN_EPS = 1e-5
COS_EPS2 = 1e-12

_cache = {}


def build_nc(nbt=BL // P):
    """Build the single-core Bass module; nbt = number of 128-row batch tiles."""
    from contextlib import ExitStack

    import concourse.bass as bass
    import concourse.mybir as mybir
    import concourse.tile as tile
    import concourse.tile_rust as tile_rust
    from concourse.masks import make_identity

    fp32 = mybir.dt.float32
    f32r = mybir.dt.float32r
    bf16 = mybir.dt.bfloat16
    i32 = mybir.dt.int32
    AF = mybir.ActivationFunctionType
    OP = mybir.AluOpType
    bl = nbt * P
    QMAGIC = 0x5F3759DF

    def r(ap):
        # reinterpret fp32 as float32r: 1 cyc/row PE streaming (vs 4 for fp32)
        return ap.bitcast(f32r)

    nc = bass.Bass()
    xd = nc.dram_tensor("x", [bl, DIM_I], fp32, kind="ExternalInput")
    hxd = nc.dram_tensor("hx", [bl, DIM_H], fp32, kind="ExternalInput")
    cxd = nc.dram_tensor("cx", [bl, DIM_H], fp32, kind="ExternalInput")
    Wd = nc.dram_tensor("W", [DIM_I + DIM_H, H4], fp32, kind="ExternalInput")
    bd = nc.dram_tensor("b", [H4], fp32, kind="ExternalInput")
    Wmd = nc.dram_tensor("Wm", [DIM_I, DIM_H], fp32, kind="ExternalInput")
    bmd = nc.dram_tensor("bm", [DIM_H], fp32, kind="ExternalInput")
    gd = nc.dram_tensor("gammas", [4, DIM_H], fp32, kind="ExternalInput")
    btd = nc.dram_tensor("betas", [4, DIM_H], fp32, kind="ExternalInput")
    hxo = nc.dram_tensor("hx_out", [bl, DIM_H], fp32, kind="ExternalOutput")
    cxo = nc.dram_tensor("cx_out", [bl, DIM_H], fp32, kind="ExternalOutput")

    def bcast_row(src_ap):
        # view an [N]-shaped AP as [P, N] with 0-step partition broadcast
        return bass.AP(
            tensor=src_ap.tensor, offset=src_ap.offset, ap=[[0, P]] + list(src_ap.ap)
        )

    def raw(inst):
        return getattr(inst, "ins", inst)

    with tile.TileContext(nc) as tc, ExitStack() as ctx:
        singles = ctx.enter_context(tc.tile_pool(name="singles", bufs=1))

        ident = singles.tile([P, P], fp32)
        make_identity(nc, ident)
        identr = singles.tile([P, P], f32r)
        nc.scalar.copy(identr, ident)
        halfc = singles.tile([P, 1], fp32)
        i_msl = nc.vector.memset(halfc, 0.5)

        # transposed activations, persistent across both phases
        xsT_all = singles.tile([P, nbt, NKB1, P], f32r)
        hxT_all = singles.tile([P, nbt, NKB1, P], f32r)

        Wv = Wd[:].rearrange("(kb p) n -> p kb n", p=P)
        Wmv = Wmd[:].rearrange("(kb p) n -> p kb n", p=P)

        def absorber(ps_tile):
            def absorb(dep_inst=None):
                """Dummy PE transpose pre-observing one semaphore so real
                matmuls never need more than one sync wait (S3_LW limit)."""
                di = nc.tensor.transpose(ps_tile, ident, ident)
                if dep_inst is not None:
                    tile_rust.add_dep_helper(
                        raw(di), raw(dep_inst), reason="absorb sem for PE"
                    )
                return di

            return absorb

        def rsqrt_dve(pool, tagp, src, eps_const=None):
            """1/sqrt(src) on DVE only: quake-III seed + one Newton step
            (worst-case rel err ~5e-4). Keeps ACT free of the sqrt table
            set so the whole kernel stays in sigmoid_and_others."""
            if eps_const is not None:
                ve = pool.tile([P, 1], fp32, tag=f"{tagp}_ve")
                nc.vector.tensor_scalar_add(ve, src, eps_const)
                src = ve
            ii = pool.tile([P, 1], i32, tag=f"{tagp}_i")
            # ~(bits >> 1), then + (MAGIC+1)  ==  MAGIC - (bits >> 1)
            nc.vector.tensor_scalar(
                ii, src.bitcast(i32), 1, -1, OP.logical_shift_right, OP.bitwise_xor
            )
            nc.vector.tensor_scalar_add(ii, ii, QMAGIC + 1)
            y0 = ii.bitcast(fp32)
            # [P,1]x[P,1] products as tensor_scalar with scalar-AP operands
            # (TS smalls are ~6x cheaper than TT smalls on DVE)
            e = pool.tile([P, 1], fp32, tag=f"{tagp}_e")
            nc.vector.tensor_scalar(e, src, y0, y0, OP.mult, OP.mult)
            nc.vector.tensor_scalar(e, e, -0.5, 1.5, OP.mult, OP.add)
            out = pool.tile([P, 1], fp32, tag=f"{tagp}_o")
            nc.vector.tensor_scalar_mul(out, e, y0)
            return out

        # ---------------- phase 1 ----------------
        with ExitStack() as p1:
            wm_pool = p1.enter_context(tc.tile_pool(name="wm", bufs=1))
            io_pool = p1.enter_context(tc.tile_pool(name="io1", bufs=2))
            sm_pool = p1.enter_context(tc.tile_pool(name="smalls1", bufs=4))
            dump_pool = p1.enter_context(tc.tile_pool(name="dump1", bufs=2))
            ps_aux = p1.enter_context(tc.tile_pool(name="psaux", bufs=3, space="PSUM"))
            ps_m1 = p1.enter_context(tc.tile_pool(name="psm1", bufs=2, space="PSUM"))

            x_pool = p1.enter_context(tc.tile_pool(name="xin", bufs=nbt))
            map_pool = p1.enter_context(tc.tile_pool(name="map1", bufs=3))
            s1_pool = p1.enter_context(tc.tile_pool(name="s1p", bufs=nbt))
            hx_all = wm_pool.tile([P, nbt, DIM_H], fp32)
            xtiles, xload, hxload = [], [], []
            for t in range(nbt):
                x_t = x_pool.tile([P, DIM_I], fp32, tag="x", name=f"x{t}")
                xtiles.append(x_t)
                xload.append(
                    nc.sync.dma_start(out=x_t, in_=xd[t * P : (t + 1) * P, :])
                )
                hxload.append(
                    nc.sync.dma_start(
                        out=hx_all[:, t], in_=hxd[t * P : (t + 1) * P, :]
                    )
                )
            wm_sb = wm_pool.tile([P, NKB1, DIM_H], f32r)
            i_wm = nc.sync.dma_start(out=wm_sb, in_=Wmv.bitcast(f32r))
            bm_rep = wm_pool.tile([P, DIM_H], f32r)
            i_bm = nc.sync.dma_start(out=bm_rep, in_=bcast_row(bmd[:]).bitcast(f32r))

            dmy = ps_aux.tile([P, P], fp32, tag="dmy", bufs=1, name="dmy")
            absorb = absorber(dmy)
            absorb()  # ident (gpsimd sem)
            absorb(i_msl)  # vector memsets
            absorb(i_bm)  # bm_rep dma queue
            absorb(i_wm)  # wm dma queue

            # -- 1a: raw transposes + mm1, PE-dense; chains come later so the
            # in-order PE stream never waits on the cosine math
            cp_insts, maps = [], []
            for t in range(nbt):
                x_t = xtiles[t]
                hx_t = hx_all[:, t]

                absorb(xload[t])
                xT_t = io_pool.tile([P, NKB1, P], f32r, tag="xT_t")
                for j in range(NKB1):
                    pt = ps_aux.tile([P, P], fp32, tag="paux", name=f"ptx{t}_{j}")
                    nc.tensor.transpose(pt, x_t[:, j * P : (j + 1) * P], ident)
                    nc.scalar.copy(xT_t[:, j, :], pt)
                absorb(hxload[t])
                for j in range(NKB1):
                    pt = ps_aux.tile([P, P], fp32, tag="paux", name=f"pth{t}_{j}")
                    nc.tensor.transpose(pt, hx_t[:, j * P : (j + 1) * P], ident)
                    nc.vector.tensor_copy(hxT_all[:, t, j, :], pt)

                # mm1: mapped = bm + x @ Wm   (psum [P, 1024], two N=512 groups)
                if t >= 2:
                    absorb(cp_insts[t - 2])  # pm slot release (DVE copy)
                pm = ps_m1.tile([P, DIM_H], fp32, tag="pm1", name=f"pm{t}")
                for nh in range(2):
                    cs = slice(nh * 512, (nh + 1) * 512)
                    nc.tensor.matmul(
                        pm[:, cs], identr, bm_rep[:, cs], start=True, stop=False
                    )
                    for kb in range(NKB1):
                        nc.tensor.matmul(
                            pm[:, cs],
                            xT_t[:, kb, :],
                            wm_sb[:, kb, cs],
                            start=False,
                            stop=(kb == NKB1 - 1),
                        )
                map_sb = map_pool.tile([P, DIM_H], fp32, tag="map_sb")
                maps.append(map_sb)
                cp_insts.append(nc.vector.tensor_copy(map_sb, pm))

            # -- 1b: cosine attention chains, all tiles (DVE/ACT only)
            s1s = []
            for t in range(nbt):
                hx_t = hx_all[:, t]
                map_sb = maps[t]
                dot_t = sm_pool.tile([P, 1], fp32, tag="dot")
                dmp0 = dump_pool.tile([P, DIM_H], fp32, tag="dump")
                nc.vector.scalar_tensor_tensor(
                    out=dmp0,
                    in0=map_sb,
                    scalar=1.0,
                    in1=hx_t,
                    op0=OP.mult,
                    op1=OP.mult,
                    accum_out=dot_t,
                )
                sqm_t = sm_pool.tile([P, 1], fp32, tag="sqm")
                dmp1 = dump_pool.tile([P, DIM_H], fp32, tag="dump")
                nc.scalar.activation(dmp1, map_sb, AF.Square, accum_out=sqm_t)
                sqh_t = sm_pool.tile([P, 1], fp32, tag="sqh")
                dmp2 = dump_pool.tile([P, DIM_H], fp32, tag="dump")
                nc.scalar.activation(dmp2, hx_t, AF.Square, accum_out=sqh_t)

                m1_t = sm_pool.tile([P, 1], fp32, tag="m1")
                nc.vector.tensor_scalar_max(m1_t, sqm_t, COS_EPS2)
                m2_t = sm_pool.tile([P, 1], fp32, tag="m2")
                nc.vector.tensor_scalar_max(m2_t, sqh_t, COS_EPS2)
                den_t = sm_pool.tile([P, 1], fp32, tag="den")
                nc.vector.tensor_scalar_mul(den_t, m1_t, m2_t)
                rinv_t = rsqrt_dve(sm_pool, "p1r", den_t)
                cos_t = sm_pool.tile([P, 1], fp32, tag="cos")
                nc.vector.tensor_scalar_mul(cos_t, dot_t, rinv_t)
                attn_t = sm_pool.tile([P, 1], fp32, tag="attn")
                nc.scalar.activation(attn_t, cos_t, AF.Sigmoid)
                s1_t = s1_pool.tile([P, 1], fp32, tag="s1", name=f"s1_{t}")
                nc.scalar.add(s1_t, attn_t, 1.0)
                s1s.append(s1_t)

            # -- 1c: x_mod = s * x in row layout (s is per-partition here),
            # then transpose the scaled rows straight into xsT_all
            for t in range(nbt):
                x_t = xtiles[t]
                nc.vector.tensor_scalar_mul(x_t, x_t, s1s[t])
                for j in range(NKB1):
                    pt = ps_aux.tile([P, P], fp32, tag="paux", name=f"pts{t}_{j}")
                    nc.tensor.transpose(pt, x_t[:, j * P : (j + 1) * P], ident)
                    nc.scalar.copy(xsT_all[:, t, j, :], pt)

        # ---------------- phase 2 ----------------
        with ExitStack() as p2:
            w_pool = p2.enter_context(tc.tile_pool(name="wch", bufs=2))
            bsl_pool = p2.enter_context(tc.tile_pool(name="bsl", bufs=2))
            gb_pool = p2.enter_context(tc.tile_pool(name="gb", bufs=1))
            iact_pool = p2.enter_context(tc.tile_pool(name="iact", bufs=nbt))
            zst_pool = p2.enter_context(tc.tile_pool(name="zst", bufs=nbt + 1))
            z3_pool = p2.enter_context(tc.tile_pool(name="z3", bufs=2))
            st_pool = p2.enter_context(tc.tile_pool(name="stats", bufs=nbt + 2))
            ga_pool = p2.enter_context(tc.tile_pool(name="gact", bufs=2))
            cx_pool = p2.enter_context(tc.tile_pool(name="cxin", bufs=3))
            sm2_pool = p2.enter_context(tc.tile_pool(name="smalls2", bufs=2))
            ps_g = p2.enter_context(tc.tile_pool(name="psg", bufs=7, space="PSUM"))

            dmy2 = ps_g.tile([P, P], fp32, tag="dmy2", bufs=1, name="dmy2")
            absorb2 = absorber(dmy2)
            sq2_pool = p2.enter_context(tc.tile_pool(name="sq2p", bufs=nbt))
            sq2s = [
                sq2_pool.tile([P, 1], fp32, tag="sq2s", name=f"sq2s{t}")
                for t in range(nbt)
            ]

            iact = [
                iact_pool.tile([P, DIM_H], fp32, tag="iact", name=f"iact{t}")
                for t in range(nbt)
            ]
            zst = [
                zst_pool.tile([P, ZW], bf16, tag="zst", name=f"zst{t}")
                for t in range(nbt)
            ]
            stats = [
                st_pool.tile([P, NCH_G, 6], fp32, tag="st", name=f"st{t}")
                for t in range(nbt)
            ]

            # gate order: i first (stored), then g (i*g), f (cx_new), o (outputs)
            for gi, func, role in (
                (0, AF.Sigmoid, "i"),
                (2, AF.Tanh, "g"),
                (1, AF.Sigmoid, "f"),
                (3, AF.Sigmoid, "o"),
            ):
                grep_t = gb_pool.tile([P, DIM_H], fp32, tag="grep", name=f"grep{gi}")
                nc.sync.dma_start(out=grep_t, in_=bcast_row(gd[gi, :]))
                brep_t = gb_pool.tile([P, DIM_H], fp32, tag="brep", name=f"brep{gi}")
                nc.sync.dma_start(out=brep_t, in_=bcast_row(btd[gi, :]))

                for c in range(NCH_G):
                    col0 = gi * DIM_H + c * CHUNK
                    wch_a = w_pool.tile(
                        [P, NKB2 // 2, CHUNK], f32r, tag="wch_a", bufs=2,
                        name=f"wcha{gi}_{c}"
                    )
                    nc.sync.dma_start(
                        out=wch_a,
                        in_=Wv[:, : NKB2 // 2, col0 : col0 + CHUNK].bitcast(f32r),
                    )
                    wch_b = w_pool.tile(
                        [P, NKB2 // 2, CHUNK], f32r, tag="wch_b", bufs=2,
                        name=f"wchb{gi}_{c}"
                    )
                    nc.sync.dma_start(
                        out=wch_b,
                        in_=Wv[:, NKB2 // 2 :, col0 : col0 + CHUNK].bitcast(f32r),
                    )
                    bsl = bsl_pool.tile(
                        [P, CHUNK], f32r, tag="bsl", name=f"bsl{gi}_{c}"
                    )
                    i_bsl = nc.sync.dma_start(
                        out=bsl, in_=bcast_row(bd[col0 : col0 + CHUNK]).bitcast(f32r)
                    )
                    absorb2(i_bsl)

                    for t in range(nbt):
                        ps = ps_g.tile(
                            [P, CHUNK], fp32, tag="pg", name=f"pg{gi}_{c}_{t}"
                        )
                        nc.tensor.matmul(ps, identr, bsl, start=True, stop=False)
                        for kb in range(NKB2):
                            lhsT = (
                                xsT_all[:, t, kb, :]
                                if kb < NKB1
                                else hxT_all[:, t, kb - NKB1, :]
                            )
                            wkb = (
                                wch_a[:, kb, :]
                                if kb < NKB2 // 2
                                else wch_b[:, kb - NKB2 // 2, :]
                            )
                            nc.tensor.matmul(
                                ps,
                                lhsT,
                                wkb,
                                start=False,
                                stop=(kb == NKB2 - 1),
                            )
                        if c < NCH_G - 1:
                            zpart = zst[t][:, c * CHUNK : (c + 1) * CHUNK]
                            nc.scalar.copy(zpart, ps)
                            nc.vector.bn_stats(stats[t][:, c, :], zpart)
                        else:
                            z3_t = z3_pool.tile(
                                [P, CHUNK], bf16, tag="z3", name=f"z3_{gi}_{t}"
                            )
                            nc.scalar.copy(z3_t, ps)
                            nc.vector.bn_stats(stats[t][:, c, :], z3_t)

                            mv_t = sm2_pool.tile([P, 2], fp32, tag="mv")
                            nc.vector.bn_aggr(mv_t, stats[t])
                            rstd_t = rsqrt_dve(
                                sm2_pool, "lnr", mv_t[:, 1:2], eps_const=LN_EPS
                            )
                            nmu_t = sm2_pool.tile([P, 1], fp32, tag="nmu")
                            nc.vector.tensor_scalar(
                                nmu_t, mv_t[:, 0:1], rstd_t, -1.0, OP.mult, OP.mult
                            )
                            if role == "i":
                                ga = iact[t]
                            else:
                                ga = ga_pool.tile(
                                    [P, DIM_H], fp32, tag="ga", name=f"ga{gi}_{t}"
                                )
                            nc.vector.tensor_scalar(
                                ga[:, 0:ZW],
                                zst[t],
                                rstd_t,
                                nmu_t,
                                OP.mult,
                                OP.add,
                            )
                            nc.vector.tensor_scalar(
                                ga[:, ZW:DIM_H],
                                z3_t,
                                rstd_t,
                                nmu_t,
                                OP.mult,
                                OP.add,
                            )
                            if role == "o":
                                nc.vector.tensor_tensor(ga, ga, grep_t, OP.mult)
                                nc.vector.tensor_tensor(ga, ga, brep_t, OP.add)
                            else:
                                nc.gpsimd.tensor_tensor(ga, ga, grep_t, OP.mult)
                                nc.gpsimd.tensor_tensor(ga, ga, brep_t, OP.add)
                            nc.scalar.activation(ga, ga, func)

                            if role == "g":
                                nc.vector.tensor_tensor(iact[t], iact[t], ga, OP.mult)
                            elif role == "f":
                                cx_t = cx_pool.tile(
                                    [P, DIM_H], fp32, tag="cx", name=f"cx{t}"
                                )
                                nc.sync.dma_start(
                                    out=cx_t, in_=cxd[t * P : (t + 1) * P, :]
                                )
                                nc.vector.tensor_tensor(cx_t, ga, cx_t, OP.mult)
                                nc.vector.tensor_tensor(iact[t], iact[t], cx_t, OP.add)
                                nc.scalar.dma_start(
                                    out=cxo[t * P : (t + 1) * P, :], in_=iact[t]
                                )
                                nc.scalar.activation(
                                    ga, iact[t], AF.Square, accum_out=sq2s[t]
                                )
                            elif role == "o":
                                tnh_t = cx_pool.tile(
                                    [P, DIM_H], fp32, tag="cx", name=f"tnh{t}"
                                )
                                nc.scalar.activation(tnh_t, iact[t], AF.Tanh)
                                # hx_new in place of tanh(cx_new)
                                hxn_t = tnh_t
                                nc.vector.tensor_tensor(hxn_t, ga, tnh_t, OP.mult)

                                # second cosine gate; dumps overwrite dead
                                # tiles (ga after hxn mult, iact after dot2);
                                # sq2 was precomputed in the f-gate
                                sq2 = sq2s[t]
                                dot2 = sm2_pool.tile([P, 1], fp32, tag="dot2")
                                nc.vector.scalar_tensor_tensor(
                                    out=iact[t],
                                    in0=hxn_t,
                                    scalar=1.0,
                                    in1=iact[t],
                                    op0=OP.mult,
                                    op1=OP.mult,
                                    accum_out=dot2,
                                )
                                sq1 = sm2_pool.tile([P, 1], fp32, tag="sq1")
                                nc.scalar.activation(
                                    ga, hxn_t, AF.Square, accum_out=sq1
                                )
                                ma = sm2_pool.tile([P, 1], fp32, tag="ma")
                                nc.vector.tensor_scalar_max(ma, sq1, COS_EPS2)
                                mb = sm2_pool.tile([P, 1], fp32, tag="mb")
                                nc.vector.tensor_scalar_max(mb, sq2, COS_EPS2)
                                dn2 = sm2_pool.tile([P, 1], fp32, tag="dn2")
                                nc.vector.tensor_scalar_mul(dn2, ma, mb)
                                rr2 = rsqrt_dve(sm2_pool, "o2r", dn2)
                                arg2 = sm2_pool.tile([P, 1], fp32, tag="arg2")
                                nc.vector.tensor_scalar(
                                    arg2, dot2, rr2, 0.5, OP.mult, OP.mult
                                )
                                co_t = sm2_pool.tile([P, 1], fp32, tag="co")
                                nc.scalar.activation(
                                    co_t, arg2, AF.Sigmoid, bias=halfc
                                )
                                nc.vector.tensor_scalar_add(co_t, co_t, 1.0)
                                nc.vector.tensor_scalar_mul(hxn_t, hxn_t, co_t)
                                nc.scalar.dma_start(
                                    out=hxo[t * P : (t + 1) * P, :], in_=hxn_t
                                )
    _split_excess_waits(nc)
    return nc


def _split_excess_waits(nc):
    """Walrus ISA structs have limited sync-wait slots (Matmult/LDW: 1,
    DMA: 2, several DVE/ACT structs: 1-2). The Tile scheduler can emit more.
    Move excess waits onto standalone EventSemaphore instructions injected
    just before the offender on the same engine."""
    import concourse.mybir as mybir

    caps = {}
    skip = {"EventSemaphore", "RegisterMove", "UnconditionalBranch"}
    n_split = 0
    for fn in nc.m.functions:
        for blk in fn.blocks:
            out = []
            changed = False
            for ins in blk.instructions:
                si = ins.sync_info
                op = ins.concise_opcode() if callable(
                    getattr(ins, "concise_opcode", None)
                ) else None
                opname = type(ins).__name__.replace("Inst", "", 1)
                if (
                    si is not None
                    and si.on_wait
                    and opname not in skip
                    and len(si.on_wait) > caps.get(opname, 1)
                ):
                    cap = caps.get(opname, 1)
                    waits = list(si.on_wait)
                    excess, keep = waits[:-cap], waits[-cap:]
                    for k, w in enumerate(excess):
                        ev = mybir.InstEventSemaphore(
                            name=f"{ins.name}-wsp{k}",
                            ins=[],
                            outs=[],
                            sync_info=mybir.SyncInfo(on_wait=[w], on_update=[]),
                        )
                        ev.engine = ins.engine
                        out.append(ev)
                        n_split += 1
                    ins.sync_info = mybir.SyncInfo(
                        on_wait=keep, on_update=list(si.on_update)
                    )
                    changed = True
                out.append(ins)
            if changed:
                blk.instructions = out
    return n_split


def _get_nc():
    if "nc" not in _cache:
        _cache["nc"] = build_nc()
    return _cache["nc"]


def kernel(x, hx, cx, W, b, Wm, bm, gammas, betas):
    from concourse.bass_utils import run_bass_kernel_spmd

    nc = _get_nc()
    x = np.ascontiguousarray(np.asarray(x, np.float32))
    hx = np.ascontiguousarray(np.asarray(hx, np.float32))
    cx = np.ascontiguousarray(np.asarray(cx, np.float32))
    shared = {
        "W": np.ascontiguousarray(np.asarray(W, np.float32)),
        "b": np.ascontiguousarray(np.asarray(b, np.float32)),
        "Wm": np.ascontiguousarray(np.asarray(Wm, np.float32)),
        "bm": np.ascontiguousarray(np.asarray(bm, np.float32)),
        "gammas": np.ascontiguousarray(np.asarray(gammas, np.float32)),
        "betas": np.ascontiguousarray(np.asarray(betas, np.float32)),
    }
    in_maps = []
    for i in range(NCORES):
        sl = slice(i * BL, (i + 1) * BL)
        in_maps.append({"x": x[sl], "hx": hx[sl], "cx": cx[sl], **shared})
    res = run_bass_kernel_spmd(nc, in_maps, list(range(NCORES)))
    hx_mod = np.concatenate([r["hx_out"] for r in res.results], axis=0)
    cx_new = np.concatenate([r["cx_out"] for r in res.results], axis=0)
    return (hx_mod, cx_new)
